# revision 37
# baseline (speedup 1.0000x reference)
"""3D Gaussian blur (kernel_size=5, sigma=1.0) on (2,1,192,256,256) f32,
distributed over 8 Trainium2 NeuronCores.

The torch kernel factors: g[i,j,l] = aD[i] * (1/5) * bW[l] -> separable into
Gaussian along D, box along H, Gaussian along W.

Per-core (2 batches x 4 D-slabs of 48):

Phase A' fuses the H box conv AND the D gauss conv into the matmul
contraction dim: stationary tiles pack (8 d-slices x 16 h-rows) into the
128 partitions, the moving operand is a constant band over
(12 output-slice slots x 20 h-out cols), PSUM accumulates across d-blocks
and h-tiles via per-element has_written semantics.  Output q[w, o, ho]
lands w-major in PSUM, evacuated to SBUF bf16 per bank as soon as that
bank's last matmul retires; 4-col region overlaps resolved by strips +
gpsimd adds.  The last d-block carries only 4 real slices (the rest was
zero pad): it is packed as (wb, sl<4, hl) in the partition dim and
contracted with K=64 matmuls, trimming input DMA by 7%.

Phase B does the W gauss conv per h-half: stationary q-tiles
[w, 128 h-chunk], moving band [w, 130], 4 matmuls per 2-slice psum bank.

Scheduling: inputs stream as one ~0.85MB DMA per h-region on the sync
HWDGE FIFO (cst first) so arrival order = consume order; warmup/filler
matmuls on the constants keep the PE HAM clock-gate at 2.4GHz through
the input-paced A' stretch; phase B hb=0 interleaves ahead of the A'
e=5..7 columns; outputs ride gpsimd SWDGE with a contiguous-per-
partition layout.
"""
import numpy as np
import ml_dtypes

import concourse.bacc as bacc
import concourse.tile as tile
from concourse import mybir
from concourse.bass_utils import run_bass_kernel_spmd

B = 2          # batch
D = 192        # depth
HW = 256       # height = width
SLAB = 48      # output slices per core
NBK = 7        # d-blocks (6 full of 8 + 1 packed of 4)
NE = 8         # h-eighths (regions of 32 h + 4 halo = 36 cols)
P = 128
N_CORES = 8
NB = 130       # pass-B band cols
XCH = 26 * 128  # per-(e) free elems: (2wb*(6bk*2tl) + 2wb*... ) see pack

F32 = mybir.dt.float32
BF16 = mybir.dt.bfloat16

# pass A' piece table: bk -> [(o_lo, o_hi, bank, start_flag)]
PIECES = {
    0: [(0, 7, 0, True)],
    1: [(4, 13, 0, False), (14, 15, 1, True)],
    2: [(12, 13, 0, False), (14, 23, 1, False)],
    3: [(20, 27, 1, False), (28, 31, 2, True)],
    4: [(28, 39, 2, False)],
    5: [(36, 41, 2, False), (42, 47, 3, True)],
    6: [(44, 47, 3, False)],
}
# bank -> (last bk writing it, first slot, n slots)
EVAC = {0: (2, 0, 14), 1: (3, 14, 14), 2: (5, 28, 14), 3: (6, 42, 6)}
W_WARM = 12    # PE warmup fillers before first A' column
W_FILL = 4     # fillers per e in the input-paced stretch


def _taps():
    c = np.arange(5, dtype=np.float64) - 2
    u = np.exp(-c * c / 2.0)   # D-axis Gaussian (sigma=1)
    v = np.exp(-c * c)         # W-axis Gaussian (sigma^2=1/2)
    aD = (u / u.sum()).astype(np.float64)
    bW = (v / v.sum()).astype(np.float64)
    return aD, bW


def _const_tensors():
    aD, bW = _taps()
    # A' band [128=(sl,hl), 12 sigma, 20 r]: aD[sl+4-sg] * 0.2 * [0<=hl+4-r<=4]
    ba = np.zeros((P, 12, 20), dtype=np.float64)
    for sl in range(8):
        for hl in range(16):
            p = sl * 16 + hl
            for sg in range(12):
                kd = sl + 4 - sg
                if not (0 <= kd <= 4):
                    continue
                for r in range(20):
                    kh = hl + 4 - r
                    if 0 <= kh <= 4:
                        ba[p, sg, r] = aD[kd] * 0.2
    # B bands [2 wb, 128, 130]
    bw = np.zeros((2, P, NB), dtype=np.float64)
    for w in range(P):
        for c in range(NB):
            k0 = w - c + 2        # wb0: wo = c
            if 0 <= k0 <= 4:
                bw[0, w, c] = bW[k0]
            k1 = w - c + 4        # wb1: w = 128+wl, wo = 126+c
            if 0 <= k1 <= 4:
                bw[1, w, c] = bW[k1]
    # bk6 band [p=wb*64+(sl<4)*16+hl, 4 sg, 20 r]: the sl<4 rows of ba
    # duplicated per wb so lhsT/rhs share a partition base (HW requires it)
    bt = np.zeros((P, 4, 20), dtype=np.float64)
    for wb in range(2):
        bt[wb * 64:(wb + 1) * 64] = ba[0:64, 0:4, :]
    return (ba.astype(ml_dtypes.bfloat16), bw.astype(ml_dtypes.bfloat16),
            bt.astype(ml_dtypes.bfloat16))


def _build_nc():
    nc = bacc.Bacc("TRN2", target_bir_lowering=False, debug=False,
                   num_devices=N_CORES)
    # x[p, e, free]: free = [wb, bk<6, tl, wc] (3072) ++ bk6 packed (256)
    # bk6 pack: partition p = wb*64 + sl*16 + hl (sl<4), free = [tl, wc]
    x_d = nc.declare_dram_parameter("x", [P, NE, XCH], BF16, isOutput=False)
    # both const bands packed into one tensor: [0:240)=ba, [240:500)=bw
    cst_d = nc.declare_dram_parameter("cst", [P, 240 + 2 * NB + 80], BF16,
                                      isOutput=False)
    # out[p=h%128, hb=h//128, o, w]  (bf16; contiguous 4KB/partition per
    # 8-slice chunk)
    out_d = nc.declare_dram_parameter("out", [P, 2, SLAB, HW], BF16,
                                      isOutput=True)

    with tile.TileContext(nc) as tc:
        with (
            tc.tile_pool(name="consts", bufs=1) as cpool,
            tc.tile_pool(name="xcols", bufs=1) as xpool,
            tc.tile_pool(name="q", bufs=1) as qpool,
            tc.tile_pool(name="ost", bufs=4) as opool,
            tc.tile_pool(name="pa", bufs=5, space="PSUM") as papool,
            tc.tile_pool(name="pb", bufs=3, space="PSUM") as pbpool,
        ):
            cst_sb = cpool.tile([P, 240 + 2 * NB + 80], BF16, tag="cst")
            # zeros tile for DMA-independent PE warmup (memset ~5.8us)
            zt = cpool.tile([P, 512], BF16, tag="zt")
            nc.gpsimd.memset(zt[:], 0.0)
            ba_sb = cst_sb[:, 0:240].rearrange("p (s r) -> p s r", s=12, r=20)
            bw_sb = cst_sb[:, 240:240 + 2 * NB].rearrange(
                "p (w c) -> p w c", w=2, c=NB)
            bt_sb = cst_sb[:, 240 + 2 * NB:].rearrange(
                "p (s r) -> p s r", s=4, r=20)

            # flat q[wp, wb, o, h 256]; halo strips staged separately
            q_sb = qpool.tile([P, 2, SLAB, HW], BF16, tag="q")
            hst = qpool.tile([P, 2, NE - 1, SLAB, 4], BF16, tag="hst")

            # one DMA per e on the sync HWDGE FIFO, in consume order.
            # cst rides between e0 and e1 so the ring never exposes the
            # small-DMA completion bubble at the head.
            xmain, xtail = {}, {}
            for e in range(NE):
                xe = xpool.tile([P, XCH], BF16, tag=f"x{e}")
                nc.sync.dma_start(xe[:], x_d[:, e])
                if e == 0:
                    nc.sync.dma_start(cst_sb[:], cst_d[:])
                xmain[e] = xe[:, 0:24 * P].rearrange(
                    "p (wb bk tl wc) -> p wb bk tl wc",
                    wb=2, bk=6, tl=2, wc=P)
                xtail[e] = xe[:, 24 * P:26 * P].rearrange(
                    "p (tl wc) -> p tl wc", tl=2, wc=P)

            ecost = {"v": 0.0, "s": 0.0}

            def _eng(sz):
                # greedy time-balance: pick the engine with less queued work.
                # constants fit from v8 trace (ACT runs hotter than spec)
                cv = ecost["v"] + sz * 1.0 + 170
                cs = ecost["s"] + sz * 0.90 + 230
                if cv <= cs:
                    ecost["v"] = cv
                    return nc.vector.tensor_copy
                ecost["s"] = cs
                return nc.scalar.copy

            def _filler(n, src=None):
                src = cst_sb if src is None else src
                for _ in range(n):
                    fps = pbpool.tile([P, 512], F32, tag="pb")
                    nc.tensor.matmul(fps[:, 0:500], src[:, 0:P],
                                     src[:, 0:500], start=True, stop=True,
                                     skip_group_check=True)

            # ---------------- phase A' column ----------------
            def a_col(e, wb):
                pa_t = []
                for bank in range(4):
                    pat = papool.tile([P, 512], F32, tag="pa")
                    pa_t.append(pat)
                c_lo = 2 if e == 0 else 0
                c_hi = 34 if e == 7 else 32
                h0 = 32 * e - 2

                def _evac_strip(bank):
                    _, slot0, nsl = EVAC[bank]
                    src = pa_t[bank][:, 0:nsl * 36].rearrange(
                        "p (s c) -> p s c", s=nsl, c=36)
                    dst = q_sb[:, wb, slot0:slot0 + nsl,
                               h0 + c_lo:h0 + c_hi]
                    _eng(nsl * (c_hi - c_lo))(dst, src[:, :, c_lo:c_hi])
                    if e != 7:
                        # save psum cols 32:36 (ho 32e+30..33) for the
                        # boundary add into region e+1
                        _eng(nsl * 4)(hst[:, wb, e, slot0:slot0 + nsl, :],
                                      src[:, :, 32:36])

                for bk in range(NBK):
                    for tl in range(2):
                        r0 = 2 if (e == 0 and tl == 0) else 0
                        r1 = 18 if (e == 7 and tl == 1) else 20
                        for (olo, ohi, bank, st) in PIECES[bk]:
                            ns = ohi - olo + 1
                            sg0 = olo - (8 * bk - 4)
                            s0 = olo % 14
                            if bk < 6:
                                lhsT = xmain[e][:, wb, bk, tl, :]
                                rhs = ba_sb[:, sg0:sg0 + ns, r0:r1]
                            else:
                                # K=64 block: both operands at base wb*64
                                lhsT = xtail[e][wb * 64:(wb + 1) * 64, tl, :]
                                rhs = bt_sb[wb * 64:(wb + 1) * 64,
                                            sg0:sg0 + ns, r0:r1]
                            sub = pa_t[bank][:, s0 * 36:(s0 + ns) * 36]
                            sub = sub.rearrange("p (s c) -> p s c",
                                                s=ns, c=36)
                            nc.tensor.matmul(
                                sub[:, :, tl * 16 + r0: tl * 16 + r1],
                                lhsT, rhs,
                                start=(st and tl == 0), stop=True,
                                skip_group_check=True)
                    # evac each bank as soon as its last matmul retired
                    for bank, (lastbk, _, _) in EVAC.items():
                        if bk == lastbk:
                            _evac_strip(bank)
                # boundary add: ho 32e-2..32e+1 += saved strip of e-1
                if e >= 1:
                    dst = q_sb[:, wb, :, h0:h0 + 4]
                    nc.gpsimd.tensor_add(dst, hst[:, wb, e - 1, :, :], dst)

            # ---------------- phase B (per hb, groups of 2 o) --------
            ost_state = {"n": 0, "dma": 0}

            def b_group(hb, g, pool, csz=6):
                """One psum bank: output slices (2g, 2g+1) of h-half hb.
                csz = groups per output DMA chunk."""
                ps = pool.tile([P, 2, HW], F32,
                               tag="pa" if pool is papool else "pb")
                for i in range(2):
                    o = 2 * g + i
                    for wb in range(2):
                        c0 = 0 if wb == 0 else HW - NB
                        nc.tensor.matmul(
                            ps[:, i, c0:c0 + NB],
                            q_sb[:, wb, o, P * hb:P * hb + P],
                            bw_sb[:, wb, :],
                            start=(i == 0 and wb == 0),
                            stop=(i == 1 and wb == 1),
                            skip_group_check=True)
                k = ost_state["n"]
                if k == 0:
                    ost = opool.tile([P, 2 * csz, HW], BF16, tag="ost")
                    ost_state["t"] = ost
                ost = ost_state["t"]
                _eng(2 * HW)(ost[:, 2 * k:2 * k + 2], ps[:])
                if k == csz - 1:
                    o0 = 2 * (g - csz + 1)
                    # alternate the two HWDGE rings so per-DMA completion
                    # receipts pipeline across FIFOs (SWDGE serializes)
                    ring = nc.sync if ost_state["dma"] % 2 == 0 \
                        else nc.scalar
                    ost_state["dma"] += 1
                    ring.dma_start(out_d[:, hb, o0:o0 + 2 * csz], ost[:])
                    ost_state["n"] = 0
                else:
                    ost_state["n"] = k + 1

            # ---------------- emission schedule ----------------
            # e<=3: input-paced, pad PE with fillers.  From e=4 on, B hb=0
            # work is ready (needs only e<=4): use it as the filler, which
            # also starts the output-DMA stream ~8us earlier.  The late A'
            # columns run as soon as their input lands so hb1 (which needs
            # a_col(7,1)) isn't pushed out.
            _filler(W_WARM, src=zt)   # zeros-fed: no DMA dependency
            for e in range(4):
                a_col(e, 0)
                a_col(e, 1)
                _filler(W_FILL)
            g = 0
            for e in range(4, NE):
                a_col(e, 0)
                a_col(e, 1)
                nb = {4: 4, 5: 2, 6: 2, 7: 0}[e]
                for _ in range(nb):
                    b_group(0, g, pbpool)
                    g += 1
            while g < SLAB // 2:
                b_group(0, g, pbpool)
                g += 1
            # B hb=1 (needs all of A'); pa banks are free now -> 5-deep
            # rotation
            for g in range(SLAB // 2):
                b_group(1, g, papool, csz=6)

    nc.compile()
    return nc


_NC_CACHE = {}


def _get_nc():
    if "nc" not in _NC_CACHE:
        _NC_CACHE["nc"] = _build_nc()
    return _NC_CACHE["nc"]


def kernel(x, kernel_size, _trace=False, _trace_kwargs=None):
    """x: (2, 1, 192, 256, 256) float32; kernel_size: 5. Returns same shape."""
    assert int(kernel_size) == 5, "kernel hardcodes kernel_size=5"
    x = np.asarray(x)
    assert x.shape == (B, 1, D, HW, HW), x.shape
    in_dtype = x.dtype

    nc = _get_nc()
    ba, bw, bt = _const_tensors()
    cst = np.concatenate(
        [np.asarray(ba).reshape(P, 240),
         np.asarray(bw).transpose(1, 0, 2).reshape(P, 2 * NB),
         np.asarray(bt).reshape(P, 80)],
        axis=1)

    xb = np.asarray(x[:, 0]).astype(ml_dtypes.bfloat16)

    in_maps = []
    for c in range(N_CORES):
        b, j = divmod(c, 4)
        lo = SLAB * j - 2
        xp = np.zeros((52, HW, HW), dtype=ml_dtypes.bfloat16)
        g0, g1 = max(0, lo), min(D, lo + 52)
        xp[g0 - lo:g1 - lo] = xb[b, g0:g1]
        # main: t 0..47 -> [bk, sl, e, tl, hl, wb, wc] -> [sl hl, e, wb bk tl wc]
        mn = xp[0:48].reshape(6, 8, NE, 2, 16, 2, P).transpose(
            1, 4, 2, 5, 0, 3, 6)
        mn = np.ascontiguousarray(mn).reshape(P, NE, 24 * P)
        # tail: t 48..51 -> [sl4, e, tl, hl, wb, wc] -> [(wb sl hl), e, tl wc]
        tl_ = xp[48:52].reshape(4, NE, 2, 16, 2, P).transpose(4, 0, 3, 1, 2, 5)
        tl_ = np.ascontiguousarray(tl_).reshape(P, NE, 2 * P)
        sw = np.ascontiguousarray(np.concatenate([mn, tl_], axis=2))
        in_maps.append({"x": sw, "cst": cst})

    res = run_bass_kernel_spmd(
        nc, in_maps, core_ids=list(range(N_CORES)),
        trace=_trace, **(_trace_kwargs or {}))

    out = np.empty((B, 1, D, HW, HW), dtype=np.float32)
    for c in range(N_CORES):
        b, j = divmod(c, 4)
        r = np.asarray(res.results[c]["out"]).astype(np.float32)
        # r[p, hb, o, w]: h = hb*128 + p
        out[b, 0, j * SLAB:(j + 1) * SLAB] = (
            r.transpose(2, 1, 0, 3).reshape(SLAB, HW, HW))

    if _trace:
        kernel._last_result = res
    return out.astype(in_dtype, copy=False)


# revision 38
# speedup vs baseline: 1.0722x; 1.0722x over previous
"""3D Gaussian blur (kernel_size=5, sigma=1.0) on (2,1,192,256,256) f32,
distributed over 8 Trainium2 NeuronCores.

The torch kernel factors: g[i,j,l] = aD[i] * (1/5) * bW[l] -> separable into
Gaussian along D, box along H, Gaussian along W.

Per-core (2 batches x 4 D-slabs of 48):

Phase A' fuses the H box conv AND the D gauss conv into the matmul
contraction dim: stationary tiles pack (8 d-slices x 16 h-rows) into the
128 partitions, the moving operand is a constant band over
(12 output-slice slots x 20 h-out cols), PSUM accumulates across d-blocks
and h-tiles via per-element has_written semantics.  Output q[w, o, ho]
lands w-major in PSUM, evacuated to SBUF bf16 per bank as soon as that
bank's last matmul retires; 4-col region overlaps resolved by strips +
gpsimd adds.  The last d-block carries only 4 real slices (the rest was
zero pad): it is packed as (wb, sl<4, hl) in the partition dim and
contracted with K=64 matmuls, trimming input DMA by 7%.

Phase B does the W gauss conv per h-half: stationary q-tiles
[w, 128 h-chunk], moving band [w, 130], 4 matmuls per 2-slice psum bank.

Scheduling: inputs stream as one ~0.85MB DMA per h-region on the sync
HWDGE FIFO (cst first) so arrival order = consume order; warmup/filler
matmuls on the constants keep the PE HAM clock-gate at 2.4GHz through
the input-paced A' stretch; phase B hb=0 interleaves ahead of the A'
e=5..7 columns; outputs ride gpsimd SWDGE with a contiguous-per-
partition layout.
"""
import numpy as np
import ml_dtypes

import concourse.bacc as bacc
import concourse.tile as tile
from concourse import mybir
from concourse.bass_utils import run_bass_kernel_spmd

B = 2          # batch
D = 192        # depth
HW = 256       # height = width
SLAB = 48      # output slices per core
NBK = 7        # d-blocks (6 full of 8 + 1 packed of 4)
NE = 8         # h-eighths (regions of 32 h + 4 halo = 36 cols)
P = 128
N_CORES = 8
NB = 130       # pass-B band cols
XCH = 26 * 128  # per-(e) free elems: (2wb*(6bk*2tl) + 2wb*... ) see pack

F32 = mybir.dt.float32
BF16 = mybir.dt.bfloat16

# pass A' piece table: bk -> [(o_lo, o_hi, bank, start_flag)]
PIECES = {
    0: [(0, 7, 0, True)],
    1: [(4, 13, 0, False), (14, 15, 1, True)],
    2: [(12, 13, 0, False), (14, 23, 1, False)],
    3: [(20, 27, 1, False), (28, 31, 2, True)],
    4: [(28, 39, 2, False)],
    5: [(36, 41, 2, False), (42, 47, 3, True)],
    6: [(44, 47, 3, False)],
}
# bank -> (last bk writing it, first slot, n slots)
EVAC = {0: (2, 0, 14), 1: (3, 14, 14), 2: (5, 28, 14), 3: (6, 42, 6)}
W_WARM = 12    # PE warmup fillers before first A' column
W_FILL = 4     # fillers per e in the input-paced stretch


def _taps():
    c = np.arange(5, dtype=np.float64) - 2
    u = np.exp(-c * c / 2.0)   # D-axis Gaussian (sigma=1)
    v = np.exp(-c * c)         # W-axis Gaussian (sigma^2=1/2)
    aD = (u / u.sum()).astype(np.float64)
    bW = (v / v.sum()).astype(np.float64)
    return aD, bW


def _const_tensors():
    aD, bW = _taps()
    # A' band [128=(sl,hl), 12 sigma, 20 r]: aD[sl+4-sg] * 0.2 * [0<=hl+4-r<=4]
    ba = np.zeros((P, 12, 20), dtype=np.float64)
    for sl in range(8):
        for hl in range(16):
            p = sl * 16 + hl
            for sg in range(12):
                kd = sl + 4 - sg
                if not (0 <= kd <= 4):
                    continue
                for r in range(20):
                    kh = hl + 4 - r
                    if 0 <= kh <= 4:
                        ba[p, sg, r] = aD[kd] * 0.2
    # B bands [2 wb, 128, 130]
    bw = np.zeros((2, P, NB), dtype=np.float64)
    for w in range(P):
        for c in range(NB):
            k0 = w - c + 2        # wb0: wo = c
            if 0 <= k0 <= 4:
                bw[0, w, c] = bW[k0]
            k1 = w - c + 4        # wb1: w = 128+wl, wo = 126+c
            if 0 <= k1 <= 4:
                bw[1, w, c] = bW[k1]
    # bk6 band [p=wb*64+(sl<4)*16+hl, 4 sg, 20 r]: the sl<4 rows of ba
    # duplicated per wb so lhsT/rhs share a partition base (HW requires it)
    bt = np.zeros((P, 4, 20), dtype=np.float64)
    for wb in range(2):
        bt[wb * 64:(wb + 1) * 64] = ba[0:64, 0:4, :]
    return (ba.astype(ml_dtypes.bfloat16), bw.astype(ml_dtypes.bfloat16),
            bt.astype(ml_dtypes.bfloat16))


def _build_nc():
    nc = bacc.Bacc("TRN2", target_bir_lowering=False, debug=False,
                   num_devices=N_CORES)
    # x[p, e, free]: free = [wb, bk<6, tl, wc] (3072) ++ bk6 packed (256)
    # bk6 pack: partition p = wb*64 + sl*16 + hl (sl<4), free = [tl, wc]
    x_d = nc.declare_dram_parameter("x", [P, NE, XCH], BF16, isOutput=False)
    # both const bands packed into one tensor: [0:240)=ba, [240:500)=bw
    cst_d = nc.declare_dram_parameter("cst", [P, 240 + 2 * NB + 80], BF16,
                                      isOutput=False)
    # out[p=h%128, hb=h//128, o, w]  (bf16; contiguous 4KB/partition per
    # 8-slice chunk)
    out_d = nc.declare_dram_parameter("out", [P, 2, SLAB, HW], BF16,
                                      isOutput=True)

    with tile.TileContext(nc) as tc:
        with (
            tc.tile_pool(name="consts", bufs=1) as cpool,
            tc.tile_pool(name="xcols", bufs=1) as xpool,
            tc.tile_pool(name="q", bufs=1) as qpool,
            tc.tile_pool(name="ost", bufs=4) as opool,
            tc.tile_pool(name="pa", bufs=5, space="PSUM") as papool,
            tc.tile_pool(name="pb", bufs=3, space="PSUM") as pbpool,
        ):
            cst_sb = cpool.tile([P, 240 + 2 * NB + 80], BF16, tag="cst")
            # zeros tile for DMA-independent PE warmup (memset ~5.8us)
            zt = cpool.tile([P, 512], BF16, tag="zt")
            nc.gpsimd.memset(zt[:], 0.0)
            ba_sb = cst_sb[:, 0:240].rearrange("p (s r) -> p s r", s=12, r=20)
            bw_sb = cst_sb[:, 240:240 + 2 * NB].rearrange(
                "p (w c) -> p w c", w=2, c=NB)
            bt_sb = cst_sb[:, 240 + 2 * NB:].rearrange(
                "p (s r) -> p s r", s=4, r=20)

            # flat q[wp, wb, o, h 256]; halo strips staged separately
            q_sb = qpool.tile([P, 2, SLAB, HW], BF16, tag="q")
            hst = qpool.tile([P, 2, NE - 1, SLAB, 4], BF16, tag="hst")

            # one DMA per e on the sync HWDGE FIFO, in consume order.
            # cst rides between e0 and e1 so the ring never exposes the
            # small-DMA completion bubble at the head.
            xmain, xtail = {}, {}
            for e in range(NE):
                xe = xpool.tile([P, XCH], BF16, tag=f"x{e}")
                nc.sync.dma_start(xe[:], x_d[:, e])
                if e == 0:
                    nc.sync.dma_start(cst_sb[:], cst_d[:])
                xmain[e] = xe[:, 0:24 * P].rearrange(
                    "p (wb bk tl wc) -> p wb bk tl wc",
                    wb=2, bk=6, tl=2, wc=P)
                xtail[e] = xe[:, 24 * P:26 * P].rearrange(
                    "p (tl wc) -> p tl wc", tl=2, wc=P)

            ecost = {"v": 0.0, "s": 0.0}

            def _eng(sz):
                # greedy time-balance: pick the engine with less queued work.
                # constants fit from v8 trace (ACT runs hotter than spec)
                cv = ecost["v"] + sz * 1.0 + 170
                cs = ecost["s"] + sz * 0.90 + 230
                if cv <= cs:
                    ecost["v"] = cv
                    return nc.vector.tensor_copy
                ecost["s"] = cs
                return nc.scalar.copy

            def _filler(n, src=None):
                src = cst_sb if src is None else src
                for _ in range(n):
                    fps = pbpool.tile([P, 512], F32, tag="pb")
                    nc.tensor.matmul(fps[:, 0:500], src[:, 0:P],
                                     src[:, 0:500], start=True, stop=True,
                                     skip_group_check=True)

            # ---------------- phase A' column ----------------
            def a_col(e, wb):
                pa_t = []
                for bank in range(4):
                    pat = papool.tile([P, 512], F32, tag="pa")
                    pa_t.append(pat)
                c_lo = 2 if e == 0 else 0
                c_hi = 34 if e == 7 else 32
                h0 = 32 * e - 2

                def _evac_strip(bank):
                    _, slot0, nsl = EVAC[bank]
                    src = pa_t[bank][:, 0:nsl * 36].rearrange(
                        "p (s c) -> p s c", s=nsl, c=36)
                    dst = q_sb[:, wb, slot0:slot0 + nsl,
                               h0 + c_lo:h0 + c_hi]
                    _eng(nsl * (c_hi - c_lo))(dst, src[:, :, c_lo:c_hi])
                    if e != 7:
                        # save psum cols 32:36 (ho 32e+30..33) for the
                        # boundary add into region e+1
                        _eng(nsl * 4)(hst[:, wb, e, slot0:slot0 + nsl, :],
                                      src[:, :, 32:36])

                for bk in range(NBK):
                    for tl in range(2):
                        r0 = 2 if (e == 0 and tl == 0) else 0
                        r1 = 18 if (e == 7 and tl == 1) else 20
                        for (olo, ohi, bank, st) in PIECES[bk]:
                            ns = ohi - olo + 1
                            sg0 = olo - (8 * bk - 4)
                            s0 = olo % 14
                            if bk < 6:
                                lhsT = xmain[e][:, wb, bk, tl, :]
                                rhs = ba_sb[:, sg0:sg0 + ns, r0:r1]
                            else:
                                # K=64 block: both operands at base wb*64
                                lhsT = xtail[e][wb * 64:(wb + 1) * 64, tl, :]
                                rhs = bt_sb[wb * 64:(wb + 1) * 64,
                                            sg0:sg0 + ns, r0:r1]
                            sub = pa_t[bank][:, s0 * 36:(s0 + ns) * 36]
                            sub = sub.rearrange("p (s c) -> p s c",
                                                s=ns, c=36)
                            nc.tensor.matmul(
                                sub[:, :, tl * 16 + r0: tl * 16 + r1],
                                lhsT, rhs,
                                start=(st and tl == 0), stop=True,
                                skip_group_check=True)
                    # evac each bank as soon as its last matmul retired
                    for bank, (lastbk, _, _) in EVAC.items():
                        if bk == lastbk:
                            _evac_strip(bank)
                # boundary add: ho 32e-2..32e+1 += saved strip of e-1
                if e >= 1:
                    dst = q_sb[:, wb, :, h0:h0 + 4]
                    nc.gpsimd.tensor_add(dst, hst[:, wb, e - 1, :, :], dst)

            # ---------------- phase B (per hb, groups of 2 o) --------
            ost_state = {"n": 0, "dma": 0}

            def b_group(hb, g, pool, csz=6):
                """One psum bank: output slices (2g, 2g+1) of h-half hb.
                csz = groups per output DMA chunk."""
                ps = pool.tile([P, 2, HW], F32,
                               tag="pa" if pool is papool else "pb")
                for i in range(2):
                    o = 2 * g + i
                    for wb in range(2):
                        c0 = 0 if wb == 0 else HW - NB
                        nc.tensor.matmul(
                            ps[:, i, c0:c0 + NB],
                            q_sb[:, wb, o, P * hb:P * hb + P],
                            bw_sb[:, wb, :],
                            start=(i == 0 and wb == 0),
                            stop=(i == 1 and wb == 1),
                            skip_group_check=True)
                k = ost_state["n"]
                if k == 0:
                    ost = opool.tile([P, 2 * csz, HW], BF16, tag="ost")
                    ost_state["t"] = ost
                ost = ost_state["t"]
                _eng(2 * HW)(ost[:, 2 * k:2 * k + 2], ps[:])
                if k == csz - 1:
                    o0 = 2 * (g - csz + 1)
                    # sync ring only: a dma_start on ACT would block the
                    # in-order ACT sequencer on the shared HWDGE block,
                    # stalling the evac stream behind it
                    nc.sync.dma_start(out_d[:, hb, o0:o0 + 2 * csz], ost[:])
                    ost_state["n"] = 0
                else:
                    ost_state["n"] = k + 1

            # ---------------- emission schedule ----------------
            # e<=3: input-paced, pad PE with fillers.  From e=4 on, B hb=0
            # work is ready (needs only e<=4): use it as the filler, which
            # also starts the output-DMA stream ~8us earlier.  The late A'
            # columns run as soon as their input lands so hb1 (which needs
            # a_col(7,1)) isn't pushed out.
            _filler(W_WARM, src=zt)   # zeros-fed: no DMA dependency
            for e in range(4):
                a_col(e, 0)
                a_col(e, 1)
                _filler(W_FILL)
            g = 0
            for e in range(4, NE):
                a_col(e, 0)
                a_col(e, 1)
                nb = {4: 4, 5: 2, 6: 2, 7: 0}[e]
                for _ in range(nb):
                    b_group(0, g, pbpool)
                    g += 1
            while g < SLAB // 2:
                b_group(0, g, pbpool)
                g += 1
            # B hb=1 (needs all of A'); pa banks are free now -> 5-deep
            # rotation
            for g in range(SLAB // 2):
                b_group(1, g, papool, csz=6)

    nc.compile()
    return nc


_NC_CACHE = {}


def _get_nc():
    if "nc" not in _NC_CACHE:
        _NC_CACHE["nc"] = _build_nc()
    return _NC_CACHE["nc"]


def kernel(x, kernel_size, _trace=False, _trace_kwargs=None):
    """x: (2, 1, 192, 256, 256) float32; kernel_size: 5. Returns same shape."""
    assert int(kernel_size) == 5, "kernel hardcodes kernel_size=5"
    x = np.asarray(x)
    assert x.shape == (B, 1, D, HW, HW), x.shape
    in_dtype = x.dtype

    nc = _get_nc()
    ba, bw, bt = _const_tensors()
    cst = np.concatenate(
        [np.asarray(ba).reshape(P, 240),
         np.asarray(bw).transpose(1, 0, 2).reshape(P, 2 * NB),
         np.asarray(bt).reshape(P, 80)],
        axis=1)

    xb = np.asarray(x[:, 0]).astype(ml_dtypes.bfloat16)

    in_maps = []
    for c in range(N_CORES):
        b, j = divmod(c, 4)
        lo = SLAB * j - 2
        xp = np.zeros((52, HW, HW), dtype=ml_dtypes.bfloat16)
        g0, g1 = max(0, lo), min(D, lo + 52)
        xp[g0 - lo:g1 - lo] = xb[b, g0:g1]
        # main: t 0..47 -> [bk, sl, e, tl, hl, wb, wc] -> [sl hl, e, wb bk tl wc]
        mn = xp[0:48].reshape(6, 8, NE, 2, 16, 2, P).transpose(
            1, 4, 2, 5, 0, 3, 6)
        mn = np.ascontiguousarray(mn).reshape(P, NE, 24 * P)
        # tail: t 48..51 -> [sl4, e, tl, hl, wb, wc] -> [(wb sl hl), e, tl wc]
        tl_ = xp[48:52].reshape(4, NE, 2, 16, 2, P).transpose(4, 0, 3, 1, 2, 5)
        tl_ = np.ascontiguousarray(tl_).reshape(P, NE, 2 * P)
        sw = np.ascontiguousarray(np.concatenate([mn, tl_], axis=2))
        in_maps.append({"x": sw, "cst": cst})

    res = run_bass_kernel_spmd(
        nc, in_maps, core_ids=list(range(N_CORES)),
        trace=_trace, **(_trace_kwargs or {}))

    out = np.empty((B, 1, D, HW, HW), dtype=np.float32)
    for c in range(N_CORES):
        b, j = divmod(c, 4)
        r = np.asarray(res.results[c]["out"]).astype(np.float32)
        # r[p, hb, o, w]: h = hb*128 + p
        out[b, 0, j * SLAB:(j + 1) * SLAB] = (
            r.transpose(2, 1, 0, 3).reshape(SLAB, HW, HW))

    if _trace:
        kernel._last_result = res
    return out.astype(in_dtype, copy=False)


# revision 39
# speedup vs baseline: 1.0764x; 1.0039x over previous
"""3D Gaussian blur (kernel_size=5, sigma=1.0) on (2,1,192,256,256) f32,
distributed over 8 Trainium2 NeuronCores.

The torch kernel factors: g[i,j,l] = aD[i] * (1/5) * bW[l] -> separable into
Gaussian along D, box along H, Gaussian along W.

Per-core (2 batches x 4 D-slabs of 48):

Phase A' fuses the H box conv AND the D gauss conv into the matmul
contraction dim: stationary tiles pack (8 d-slices x 16 h-rows) into the
128 partitions, the moving operand is a constant band over
(12 output-slice slots x 20 h-out cols), PSUM accumulates across d-blocks
and h-tiles via per-element has_written semantics.  Output q[w, o, ho]
lands w-major in PSUM, evacuated to SBUF bf16 per bank as soon as that
bank's last matmul retires; 4-col region overlaps resolved by strips +
gpsimd adds.  The last d-block carries only 4 real slices (the rest was
zero pad): it is packed as (wb, sl<4, hl) in the partition dim and
contracted with K=64 matmuls, trimming input DMA by 7%.

Phase B does the W gauss conv per h-half: stationary q-tiles
[w, 128 h-chunk], moving band [w, 130], 4 matmuls per 2-slice psum bank.

Scheduling: inputs stream as one ~0.85MB DMA per h-region on the sync
HWDGE FIFO (cst first) so arrival order = consume order; warmup/filler
matmuls on the constants keep the PE HAM clock-gate at 2.4GHz through
the input-paced A' stretch; phase B hb=0 interleaves ahead of the A'
e=5..7 columns; outputs ride gpsimd SWDGE with a contiguous-per-
partition layout.
"""
import numpy as np
import ml_dtypes

import concourse.bacc as bacc
import concourse.tile as tile
from concourse import mybir
from concourse.bass_utils import run_bass_kernel_spmd

B = 2          # batch
D = 192        # depth
HW = 256       # height = width
SLAB = 48      # output slices per core
NBK = 7        # d-blocks (6 full of 8 + 1 packed of 4)
NE = 8         # h-eighths (regions of 32 h + 4 halo = 36 cols)
P = 128
N_CORES = 8
NB = 130       # pass-B band cols
XCH = 26 * 128  # per-(e) free elems: (2wb*(6bk*2tl) + 2wb*... ) see pack

F32 = mybir.dt.float32
BF16 = mybir.dt.bfloat16

# pass A' piece table: bk -> [(o_lo, o_hi, bank, start_flag)]
PIECES = {
    0: [(0, 7, 0, True)],
    1: [(4, 13, 0, False), (14, 15, 1, True)],
    2: [(12, 13, 0, False), (14, 23, 1, False)],
    3: [(20, 27, 1, False), (28, 31, 2, True)],
    4: [(28, 39, 2, False)],
    5: [(36, 41, 2, False), (42, 47, 3, True)],
    6: [(44, 47, 3, False)],
}
# bank -> (last bk writing it, first slot, n slots)
EVAC = {0: (2, 0, 14), 1: (3, 14, 14), 2: (5, 28, 14), 3: (6, 42, 6)}
W_WARM = 12    # PE warmup fillers before first A' column
W_FILL = 4     # fillers per e in the input-paced stretch


def _taps():
    c = np.arange(5, dtype=np.float64) - 2
    u = np.exp(-c * c / 2.0)   # D-axis Gaussian (sigma=1)
    v = np.exp(-c * c)         # W-axis Gaussian (sigma^2=1/2)
    aD = (u / u.sum()).astype(np.float64)
    bW = (v / v.sum()).astype(np.float64)
    return aD, bW


def _const_tensors():
    aD, bW = _taps()
    # A' band [128=(sl,hl), 12 sigma, 20 r]: aD[sl+4-sg] * 0.2 * [0<=hl+4-r<=4]
    ba = np.zeros((P, 12, 20), dtype=np.float64)
    for sl in range(8):
        for hl in range(16):
            p = sl * 16 + hl
            for sg in range(12):
                kd = sl + 4 - sg
                if not (0 <= kd <= 4):
                    continue
                for r in range(20):
                    kh = hl + 4 - r
                    if 0 <= kh <= 4:
                        ba[p, sg, r] = aD[kd] * 0.2
    # B bands [2 wb, 128, 130]
    bw = np.zeros((2, P, NB), dtype=np.float64)
    for w in range(P):
        for c in range(NB):
            k0 = w - c + 2        # wb0: wo = c
            if 0 <= k0 <= 4:
                bw[0, w, c] = bW[k0]
            k1 = w - c + 4        # wb1: w = 128+wl, wo = 126+c
            if 0 <= k1 <= 4:
                bw[1, w, c] = bW[k1]
    # bk6 band [p=wb*64+(sl<4)*16+hl, 4 sg, 20 r]: the sl<4 rows of ba
    # duplicated per wb so lhsT/rhs share a partition base (HW requires it)
    bt = np.zeros((P, 4, 20), dtype=np.float64)
    for wb in range(2):
        bt[wb * 64:(wb + 1) * 64] = ba[0:64, 0:4, :]
    return (ba.astype(ml_dtypes.bfloat16), bw.astype(ml_dtypes.bfloat16),
            bt.astype(ml_dtypes.bfloat16))


def _build_nc():
    nc = bacc.Bacc("TRN2", target_bir_lowering=False, debug=False,
                   num_devices=N_CORES)
    # x[p, e, free]: free = [wb, bk<6, tl, wc] (3072) ++ bk6 packed (256)
    # bk6 pack: partition p = wb*64 + sl*16 + hl (sl<4), free = [tl, wc]
    x_d = nc.declare_dram_parameter("x", [P, NE, XCH], BF16, isOutput=False)
    # both const bands packed into one tensor: [0:240)=ba, [240:500)=bw
    cst_d = nc.declare_dram_parameter("cst", [P, 240 + 2 * NB + 80], BF16,
                                      isOutput=False)
    # out[p=h%128, hb=h//128, o, w]  (bf16; contiguous 4KB/partition per
    # 8-slice chunk)
    out_d = nc.declare_dram_parameter("out", [P, 2, SLAB, HW], BF16,
                                      isOutput=True)

    with tile.TileContext(nc) as tc:
        with (
            tc.tile_pool(name="consts", bufs=1) as cpool,
            tc.tile_pool(name="xcols", bufs=1) as xpool,
            tc.tile_pool(name="q", bufs=1) as qpool,
            tc.tile_pool(name="ost", bufs=4) as opool,
            tc.tile_pool(name="pa", bufs=5, space="PSUM") as papool,
            tc.tile_pool(name="pb", bufs=3, space="PSUM") as pbpool,
        ):
            cst_sb = cpool.tile([P, 240 + 2 * NB + 80], BF16, tag="cst")
            # zeros tile for DMA-independent PE warmup (memset ~5.8us)
            zt = cpool.tile([P, 512], BF16, tag="zt")
            nc.gpsimd.memset(zt[:], 0.0)
            ba_sb = cst_sb[:, 0:240].rearrange("p (s r) -> p s r", s=12, r=20)
            bw_sb = cst_sb[:, 240:240 + 2 * NB].rearrange(
                "p (w c) -> p w c", w=2, c=NB)
            bt_sb = cst_sb[:, 240 + 2 * NB:].rearrange(
                "p (s r) -> p s r", s=4, r=20)

            # flat q[wp, wb, o, h 256]; halo strips staged separately
            q_sb = qpool.tile([P, 2, SLAB, HW], BF16, tag="q")
            hst = qpool.tile([P, 2, NE - 1, SLAB, 4], BF16, tag="hst")

            # one DMA per e on the sync HWDGE FIFO, in consume order.
            # cst rides between e0 and e1 so the ring never exposes the
            # small-DMA completion bubble at the head.
            xmain, xtail = {}, {}
            for e in range(NE):
                xe = xpool.tile([P, XCH], BF16, tag=f"x{e}")
                nc.sync.dma_start(xe[:], x_d[:, e])
                if e == 0:
                    nc.sync.dma_start(cst_sb[:], cst_d[:])
                xmain[e] = xe[:, 0:24 * P].rearrange(
                    "p (wb bk tl wc) -> p wb bk tl wc",
                    wb=2, bk=6, tl=2, wc=P)
                xtail[e] = xe[:, 24 * P:26 * P].rearrange(
                    "p (tl wc) -> p tl wc", tl=2, wc=P)

            ecost = {"v": 0.0, "s": 0.0}

            def _eng(sz):
                # greedy time-balance: pick the engine with less queued work.
                # constants fit from v8 trace (ACT runs hotter than spec)
                cv = ecost["v"] + sz * 1.0 + 170
                cs = ecost["s"] + sz * 0.90 + 230
                if cv <= cs:
                    ecost["v"] = cv
                    return nc.vector.tensor_copy
                ecost["s"] = cs
                return nc.scalar.copy

            def _filler(n, src=None):
                src = cst_sb if src is None else src
                for _ in range(n):
                    fps = pbpool.tile([P, 512], F32, tag="pb")
                    nc.tensor.matmul(fps[:, 0:500], src[:, 0:P],
                                     src[:, 0:500], start=True, stop=True,
                                     skip_group_check=True)

            # ---------------- phase A' column ----------------
            def a_col(e, wb):
                pa_t = []
                for bank in range(4):
                    pat = papool.tile([P, 512], F32, tag="pa")
                    pa_t.append(pat)
                c_lo = 2 if e == 0 else 0
                c_hi = 34 if e == 7 else 32
                h0 = 32 * e - 2

                def _evac_strip(bank):
                    _, slot0, nsl = EVAC[bank]
                    src = pa_t[bank][:, 0:nsl * 36].rearrange(
                        "p (s c) -> p s c", s=nsl, c=36)
                    dst = q_sb[:, wb, slot0:slot0 + nsl,
                               h0 + c_lo:h0 + c_hi]
                    _eng(nsl * (c_hi - c_lo))(dst, src[:, :, c_lo:c_hi])
                    if e != 7:
                        # save psum cols 32:36 (ho 32e+30..33) for the
                        # boundary add into region e+1
                        _eng(nsl * 4)(hst[:, wb, e, slot0:slot0 + nsl, :],
                                      src[:, :, 32:36])

                for bk in range(NBK):
                    for tl in range(2):
                        r0 = 2 if (e == 0 and tl == 0) else 0
                        r1 = 18 if (e == 7 and tl == 1) else 20
                        for (olo, ohi, bank, st) in PIECES[bk]:
                            ns = ohi - olo + 1
                            sg0 = olo - (8 * bk - 4)
                            s0 = olo % 14
                            if bk < 6:
                                lhsT = xmain[e][:, wb, bk, tl, :]
                                rhs = ba_sb[:, sg0:sg0 + ns, r0:r1]
                            else:
                                # K=64 block: both operands at base wb*64
                                lhsT = xtail[e][wb * 64:(wb + 1) * 64, tl, :]
                                rhs = bt_sb[wb * 64:(wb + 1) * 64,
                                            sg0:sg0 + ns, r0:r1]
                            sub = pa_t[bank][:, s0 * 36:(s0 + ns) * 36]
                            sub = sub.rearrange("p (s c) -> p s c",
                                                s=ns, c=36)
                            nc.tensor.matmul(
                                sub[:, :, tl * 16 + r0: tl * 16 + r1],
                                lhsT, rhs,
                                start=(st and tl == 0), stop=True,
                                skip_group_check=True)
                    # evac each bank as soon as its last matmul retired
                    for bank, (lastbk, _, _) in EVAC.items():
                        if bk == lastbk:
                            _evac_strip(bank)
                # boundary add: ho 32e-2..32e+1 += saved strip of e-1
                if e >= 1:
                    dst = q_sb[:, wb, :, h0:h0 + 4]
                    nc.gpsimd.tensor_add(dst, hst[:, wb, e - 1, :, :], dst)

            # ---------------- phase B (per hb, groups of 2 o) --------
            ost_state = {"n": 0, "dma": 0}

            def b_group(hb, g, pool, csz=6):
                """One psum bank: output slices (2g, 2g+1) of h-half hb.
                csz = groups per output DMA chunk."""
                ps = pool.tile([P, 2, HW], F32,
                               tag="pa" if pool is papool else "pb")
                for i in range(2):
                    o = 2 * g + i
                    for wb in range(2):
                        c0 = 0 if wb == 0 else HW - NB
                        nc.tensor.matmul(
                            ps[:, i, c0:c0 + NB],
                            q_sb[:, wb, o, P * hb:P * hb + P],
                            bw_sb[:, wb, :],
                            start=(i == 0 and wb == 0),
                            stop=(i == 1 and wb == 1),
                            skip_group_check=True)
                k = ost_state["n"]
                if k == 0:
                    ost = opool.tile([P, 2 * csz, HW], BF16, tag="ost")
                    ost_state["t"] = ost
                ost = ost_state["t"]
                _eng(2 * HW)(ost[:, 2 * k:2 * k + 2], ps[:])
                if k == csz - 1:
                    o0 = 2 * (g - csz + 1)
                    # sync ring only: a dma_start on ACT would block the
                    # in-order ACT sequencer on the shared HWDGE block,
                    # stalling the evac stream behind it
                    nc.sync.dma_start(out_d[:, hb, o0:o0 + 2 * csz], ost[:])
                    ost_state["n"] = 0
                else:
                    ost_state["n"] = k + 1

            # ---------------- emission schedule ----------------
            # e<=3: input-paced, pad PE with fillers.  From e=4 on, B hb=0
            # work is ready (needs only e<=4): use it as the filler, which
            # also starts the output-DMA stream ~8us earlier.  The late A'
            # columns run as soon as their input lands so hb1 (which needs
            # a_col(7,1)) isn't pushed out.
            _filler(W_WARM, src=zt)   # zeros-fed: no DMA dependency
            for e in range(4):
                a_col(e, 0)
                a_col(e, 1)
                _filler(W_FILL)
            g = 0
            for e in range(4, NE):
                a_col(e, 0)
                a_col(e, 1)
                nb = {4: 4, 5: 2, 6: 2, 7: 0}[e]
                for _ in range(nb):
                    b_group(0, g, pbpool)
                    g += 1
            while g < SLAB // 2:
                b_group(0, g, pbpool)
                g += 1
            # B hb=1 (needs all of A'); pa banks are free now -> 5-deep
            # rotation; chunk sizes shrink at the end so the final DMA
            # (which nothing overlaps) is small
            for g in range(SLAB // 2):
                b_group(1, g, papool, csz=(6 if g < 18 else 3))

    nc.compile()
    return nc


_NC_CACHE = {}


def _get_nc():
    if "nc" not in _NC_CACHE:
        _NC_CACHE["nc"] = _build_nc()
    return _NC_CACHE["nc"]


def kernel(x, kernel_size, _trace=False, _trace_kwargs=None):
    """x: (2, 1, 192, 256, 256) float32; kernel_size: 5. Returns same shape."""
    assert int(kernel_size) == 5, "kernel hardcodes kernel_size=5"
    x = np.asarray(x)
    assert x.shape == (B, 1, D, HW, HW), x.shape
    in_dtype = x.dtype

    nc = _get_nc()
    ba, bw, bt = _const_tensors()
    cst = np.concatenate(
        [np.asarray(ba).reshape(P, 240),
         np.asarray(bw).transpose(1, 0, 2).reshape(P, 2 * NB),
         np.asarray(bt).reshape(P, 80)],
        axis=1)

    xb = np.asarray(x[:, 0]).astype(ml_dtypes.bfloat16)

    in_maps = []
    for c in range(N_CORES):
        b, j = divmod(c, 4)
        lo = SLAB * j - 2
        xp = np.zeros((52, HW, HW), dtype=ml_dtypes.bfloat16)
        g0, g1 = max(0, lo), min(D, lo + 52)
        xp[g0 - lo:g1 - lo] = xb[b, g0:g1]
        # main: t 0..47 -> [bk, sl, e, tl, hl, wb, wc] -> [sl hl, e, wb bk tl wc]
        mn = xp[0:48].reshape(6, 8, NE, 2, 16, 2, P).transpose(
            1, 4, 2, 5, 0, 3, 6)
        mn = np.ascontiguousarray(mn).reshape(P, NE, 24 * P)
        # tail: t 48..51 -> [sl4, e, tl, hl, wb, wc] -> [(wb sl hl), e, tl wc]
        tl_ = xp[48:52].reshape(4, NE, 2, 16, 2, P).transpose(4, 0, 3, 1, 2, 5)
        tl_ = np.ascontiguousarray(tl_).reshape(P, NE, 2 * P)
        sw = np.ascontiguousarray(np.concatenate([mn, tl_], axis=2))
        in_maps.append({"x": sw, "cst": cst})

    res = run_bass_kernel_spmd(
        nc, in_maps, core_ids=list(range(N_CORES)),
        trace=_trace, **(_trace_kwargs or {}))

    out = np.empty((B, 1, D, HW, HW), dtype=np.float32)
    for c in range(N_CORES):
        b, j = divmod(c, 4)
        r = np.asarray(res.results[c]["out"]).astype(np.float32)
        # r[p, hb, o, w]: h = hb*128 + p
        out[b, 0, j * SLAB:(j + 1) * SLAB] = (
            r.transpose(2, 1, 0, 3).reshape(SLAB, HW, HW))

    if _trace:
        kernel._last_result = res
    return out.astype(in_dtype, copy=False)
